# revision 26
# baseline (speedup 1.0000x reference)
"""Trainium2 Bass kernel for AttentionBasedScaleIntegrationUnit (v2).

Shapes (hardcoded): x [S=3, B=4, C=256, H=64, W=64], NH=8, HD=32.
Sharding: 8 cores; core i -> batch i//2, H-half i%2 (32 output rows each,
plus 1 halo row and 1 zero-pad row -> 34 input rows per core).

v2 changes vs baseline:
- All big matmuls in fp8e4 with DoubleRow perf mode (K=256 per instr).
  Power-of-2 scales (x*16, weights*64) keep fp8 resolution; activation
  `scale` knobs undo them exactly.
- Scalar engine runs ONLY Gelu/Copy (one act-table load).  Softmax exp
  is a Schraudolph bit-trick on DVE (tensor_scalar -> int32, bitcast).
- q-bias folded into qt via fused scalar_tensor_tensor copy; k-bias
  cancels in softmax; v-bias folded into out-proj bias (x wsum).
- A-phase in 2 token slabs with wide [*,1024/1152] activations.
- Conv on a 66-col padded grid (junk pad cols flow positionally),
  7-row chunks, contiguous DoubleRow windows.
- Score d-reduce as a TT halving tree; softmax batched per chunk;
  work spread across DVE / GPSIMD / Scalar.
"""

import math

import numpy as np
import ml_dtypes

import concourse.bass as bass
import concourse.tile as tile
import concourse.mybir as mybir
from concourse import bass_utils

S, B, C, H, W = 3, 4, 256, 64, 64
NH, HD = 8, 32
EPS = 1e-5
RH = 34                 # token rows per core (pad/halo + 32 + halo/pad)
NTOK = RH * W           # 2176 tokens per core
PW = W + 2              # padded row width 66
BROWS = RH + 2          # buf rows: guard + 34 + guard
NBP = 2384              # padded per-half int_buf stride (36*66=2376 -> 2384)
F32 = mybir.dt.float32
BF16 = mybir.dt.bfloat16
FP8 = mybir.dt.float8e4
I32 = mybir.dt.int32
BF = ml_dtypes.bfloat16
F8 = ml_dtypes.float8_e4m3   # TRN variant, max +-240

SX = 16.0    # x scale
SW = 64.0    # weight scale (all weight tensors)

# Schraudolph exp: exp(x) ~= bitcast_f32(int32(12102203.16*x + 1064866805))
EXP_A = 12102203.161561485
EXP_B = 1064866805.0

# token chunks (4,4,4,4,1 tiles) and A slabs
CHUNKS = [(0, 512), (512, 512), (1024, 512), (1536, 512), (2048, 128)]
SLABS = [(0, 256), (256, 768), (1024, 1024), (2048, 128)]
# A-phase emission: (slab, proj-unit) emitted at driver iteration key
A_AT = {-1: [(0, 0), (0, 1), (0, 2), (1, 0)], 0: [(1, 1), (1, 2)],
        1: [(2, 0)], 2: [(2, 1)], 3: [(2, 2), (3, 0)], 4: [(3, 1)],
        5: [(3, 2)]}
# conv chunks on the padded grid: (first buf row, nrows)
CONV_CHUNKS = [(2, 7), (9, 7), (16, 7), (23, 7), (30, 4)]
# conv chunks runnable after each token chunk (halo row must be written)
CONV_AFTER = {0: [], 1: [0, 1], 2: [2], 3: [3], 4: [4]}
LAST_RESULT = None


def _erf(x):
    v = np.vectorize(math.erf)
    return v(x.astype(np.float64))


def _gelu(x):
    return (x * 0.5 * (1.0 + _erf(x / math.sqrt(2.0)))).astype(np.float32)


def _fold_bn(wt, bias, bn):
    """wt [n, o, c], bias [n, o], bn [n, 4, o] -> scaled weight + eff bias."""
    g, be, mu, var = bn[:, 0], bn[:, 1], bn[:, 2], bn[:, 3]
    sc = g / np.sqrt(var + EPS)
    w_eff = wt * sc[:, :, None]
    b_eff = (bias - mu) * sc + be
    return w_eff.astype(np.float32), b_eff.astype(np.float32)


def _lhsT_pack(w_eff):
    """w_eff [n, o, c] -> [128, n*2k*256] lhsT pack (c_in on partitions)."""
    n = w_eff.shape[0]
    wt = np.transpose(w_eff, (0, 2, 1))            # [n, c_in, c_out]
    wt = wt.reshape(n, 2, 128, 256)                # [n, k, p, c_out]
    wt = np.transpose(wt, (2, 0, 1, 3))            # [p, n, k, c_out]
    return np.ascontiguousarray(wt.reshape(128, n * 2 * 256))


def _f8(x):
    return np.clip(np.asarray(x, np.float32), -240.0, 240.0).astype(F8)


def build_program():
    nc = bass.Bass("TRN2", target_bir_lowering=False, debug=False,
                   enable_asserts=False, num_devices=8)

    def din(name, shape, dt):
        return nc.dram_tensor(name, list(shape), dt, kind="ExternalInput").ap()

    xs_d = din("xs", [128, S * 2 * NTOK], FP8)      # host pre-transposed
    wallA_d = din("wallA", [128, 4608], FP8)        # wq|wk|wv (A-phase)
    wallB_d = din("wallB", [128, 7300], FP8)        # wi|wo|cw|w2|mask
    fc_d = din("fconsts", [128, 281], F32)          # bq|biq|bo|cb|wimp
    xr_d = din("xres", [128, 2 * 32 * PW], F32)
    id_d = din("ident", [128, 128], BF16)

    yout = nc.dram_tensor("yout", [2, 128, 32 * W], F32,
                          kind="ExternalOutput").ap()

    AL = mybir.AluOpType
    AF = mybir.ActivationFunctionType
    DR = mybir.MatmulPerfMode.DoubleRow

    with tile.TileContext(nc) as tc:
        with (
            tc.tile_pool(name="const", bufs=1) as cpool,
            tc.tile_pool(name="acts", bufs=1) as apool,
            tc.tile_pool(name="grp", bufs=2) as gpool,
            tc.tile_pool(name="chain", bufs=1) as hpool,
            tc.tile_pool(name="work", bufs=2) as wpool,
            tc.tile_pool(name="ps", bufs=3, space="PSUM") as ps,
            tc.tile_pool(name="tr", bufs=2, space="PSUM") as trps,
        ):
            # ---- load inputs: 5 consolidated DMAs ----
            xs_sb = cpool.tile([128, S * 2 * NTOK], FP8, tag="xs")
            for s3 in range(S):
                nc.sync.dma_start(
                    xs_sb[:, s3 * 2 * NTOK:(s3 + 1) * 2 * NTOK],
                    xs_d[:, s3 * 2 * NTOK:(s3 + 1) * 2 * NTOK])
            wallA_sb = cpool.tile([128, 4608], FP8, tag="wallA")
            nc.gpsimd.dma_start(wallA_sb[:], wallA_d[:])
            fc_sb = cpool.tile([128, 281], F32, tag="fconsts")
            nc.gpsimd.dma_start(fc_sb[:], fc_d[:])
            wallB_sb = cpool.tile([128, 7300], FP8, tag="wallB")
            nc.scalar.dma_start(wallB_sb[:], wallB_d[:])
            id_sb = cpool.tile([128, 128], BF16, tag="ident")
            nc.gpsimd.dma_start(id_sb[:], id_d[:])
            xr_sb = cpool.tile([128, 2 * 32 * PW], F32, tag="xres")
            nc.scalar.dma_start(xr_sb[:], xr_d[:])
            w_sb = {"q": wallA_sb[:, 0:1536], "k": wallA_sb[:, 1536:3072],
                    "v": wallA_sb[:, 3072:4608]}
            wi_sb = wallB_sb[:, 0:1536]
            wo_ap = wallB_sb[:, 1536:2048]
            cw_sb = wallB_sb[:, 2048:6656]
            w2_ap = wallB_sb[:, 6656:7168]
            mask_sb = wallB_sb[:, 7168:7300]
            bq_sb = fc_sb[:, 0:18]
            biq_ap = fc_sb[:, 18:274]
            bo_sb = fc_sb[:, 274:276]
            cb_sb = fc_sb[:, 276:278]
            wimp_ap = fc_sb[:, 278:281]

            int_buf = apool.tile([128, 2 * NBP], FP8, tag="ib")
            nc.gpsimd.memset(int_buf[:], 0.0)

            # qy activations: per proj [128, 3s x 2m x 2176] fp8
            qy = {p: apool.tile([128, S * 2 * NTOK], FP8, tag=f"qy{p}",
                                name=f"qy{p}")
                  for p in ("q", "k", "v")}

            # ---- A phase: qkv proj + BN + gelu, per (slab, proj) unit ----
            def emit_A_unit(si, pi):
                c0, cn = SLABS[si]
                p = ("q", "k", "v")[pi]
                for s in range(S):
                    for m in range(2):
                        pa = ps.tile([128, cn], F32, tag="ps", name="pa")
                        lhs = w_sb[p][:, s * 512:(s + 1) * 512] \
                            .rearrange("p (k c) -> p k c", k=2) \
                            [:, :, m * 128:(m + 1) * 128]
                        for n0 in range(0, cn, 512):
                            nn = min(512, cn - n0)
                            rhs = xs_sb[:, s * 2 * NTOK:(s + 1) * 2 * NTOK] \
                                .rearrange("p (k n) -> p k n", k=2) \
                                [:, :, c0 + n0:c0 + n0 + nn]
                            nc.tensor.matmul(pa[:, n0:n0 + nn], lhs, rhs,
                                             start=True, stop=True,
                                             perf_mode=DR)
                        nc.scalar.activation(
                            qy[p][:, (s * 2 + m) * NTOK + c0:
                                  (s * 2 + m) * NTOK + c0 + cn],
                            pa[:], AF.Gelu,
                            bias=bq_sb[:, ((pi * 3 + s) * 2 + m):
                                       ((pi * 3 + s) * 2 + m + 1)],
                            scale=1.0 / (SX * SW))

            # ---- conv + 1x1 + residual emitter (padded grid) ----
            def emit_conv_chunk(cci):
                r0, nr = CONV_CHUNKS[cci]
                nn = nr * PW
                yc = wpool.tile([128, 2 * 464], FP8, tag="yc", name="yc",
                                bufs=2)
                for m in range(2):
                    pc = ps.tile([128, nn], F32, tag="ps", name="pc")
                    for tap in range(9):
                        dh, dw = tap // 3 - 1, tap % 3 - 1
                        lhs = cw_sb[:, tap * 512:(tap + 1) * 512] \
                            .rearrange("p (k c) -> p k c", k=2) \
                            [:, :, m * 128:(m + 1) * 128]
                        base = (r0 + dh) * PW + dw
                        rhs = int_buf[:].rearrange("p (k n) -> p k n", k=2) \
                            [:, :, base:base + nn]
                        nc.tensor.matmul(pc[:], lhs, rhs,
                                         start=(tap == 0), stop=(tap == 8),
                                         perf_mode=DR)
                    nc.scalar.activation(
                        yc[:, m * 464:m * 464 + nn], pc[:], AF.Gelu,
                        bias=cb_sb[:, m:m + 1], scale=1.0 / (SW * SW))
                for m2 in range(2):
                    pf = ps.tile([128, nn], F32, tag="ps", name="pf")
                    lhs = w2_ap.rearrange("p (k m c) -> p k m c", k=2, m=2) \
                        [:, :, m2]
                    rhs = yc[:].rearrange("p (k n) -> p k n", k=2)[:, :, :nn]
                    nc.tensor.matmul(pf[:], lhs, rhs, start=True, stop=True,
                                     perf_mode=DR)
                    ot = wpool.tile([128, 462], F32, tag="ot", name="ot",
                                    bufs=3)
                    nc.vector.scalar_tensor_tensor(
                        ot[:, :nn], pf[:], 1.0 / SW,
                        xr_sb[:, m2 * 32 * PW + (r0 - 2) * PW:
                              m2 * 32 * PW + (r0 - 2) * PW + nn],
                        AL.mult, AL.add)
                    nc.sync.dma_start(
                        yout[m2][:, (r0 - 2) * W:(r0 - 2) * W + nr * W],
                        ot[:, :nn].rearrange("p (r w) -> p r w", w=PW)
                        [:, :, 1:1 + W])

            # ---- in-proj + attention: skewed 3-stage pipeline ----
            # groups of 2 tiles (256 tokens); group 8 is the single tail tile
            GROUPS = [(g * 256, 256, 2) for g in range(8)] + [(2048, 128, 1)]
            CONV_AFTER_G = {2: [0], 3: [1], 5: [2], 7: [3], 8: [4]}

            def g_tiles(tag, pool, width, dt, bufs=2):
                return [pool.tile([128, width], dt, tag=f"{tag}{i}",
                                  name=f"{tag}{i}", bufs=1)
                        for i in range(bufs)]

            qtg_b = g_tiles("qtg", gpool, 1536, BF16)
            ktg_b = g_tiles("ktg", gpool, 1536, BF16)
            vtg_b = g_tiles("vtg", gpool, 1536, BF16, bufs=3)
            prod_b = g_tiles("prod", hpool, 4608, BF16)
            r16_b = g_tiles("r16", hpool, 2304, BF16)
            r8_b = g_tiles("r8", hpool, 1152, BF16)
            r4_b = g_tiles("r4", hpool, 576, BF16)
            r2_b = g_tiles("r2", hpool, 288, BF16)
            sc_b = g_tiles("sc", gpool, 144, BF16)
            exi_b = g_tiles("exi", hpool, 144, I32)
            den_b = g_tiles("den", hpool, 48, F32)
            dr_b = g_tiles("dr", hpool, 48, F32)
            rr_b_ = g_tiles("rr", hpool, 48, F32)
            zt_b = g_tiles("zt", hpool, 144, F32)
            zb_b_ = g_tiles("zb", gpool, 48, BF16)
            p3_b = g_tiles("p3", hpool, 1536, BF16)
            c01_b = g_tiles("c01", hpool, 512, BF16)
            comb_b = g_tiles("comb", gpool, 512, BF16)
            ctch_b = g_tiles("ctch", wpool, 512, FP8)

            def halve(dst, src_, nblk, d, eng):
                sv = src_.rearrange("p (b e) -> p b e", b=nblk)
                eng.tensor_tensor(
                    dst.rearrange("p (b e) -> p b e", b=nblk),
                    sv[:, :, 0:d // 2], sv[:, :, d // 2:d], AL.add)

            def stage1(g):
                t0g, gn, ng = GROUPS[g]
                qtg, ktg, vtg = qtg_b[g % 2], ktg_b[g % 2], vtg_b[g % 3]
                prod = prod_b[g % 2]
                for it in range(ng):
                    t0 = t0g + it * 128
                    pj = {}
                    for pi, p in enumerate(("q", "k", "v")):
                        pjp = ps.tile([128, 768], F32, tag="ps",
                                      name=f"pj{p}")
                        for s in range(S):
                            lhs = qy[p][:, s * 2 * NTOK:(s + 1) * 2 * NTOK] \
                                .rearrange("p (m n) -> p m n", m=2) \
                                [:, :, t0:t0 + 128]
                            rhs = wi_sb[:, pi * 512:(pi + 1) * 512] \
                                .rearrange("p (m c) -> p m c", m=2)
                            nc.tensor.matmul(pjp[:, s * 256:(s + 1) * 256],
                                             lhs, rhs, start=True, stop=True,
                                             perf_mode=DR)
                        pj[p] = pjp
                    nc.vector.scalar_tensor_tensor(
                        qtg[:, it * 768:(it + 1) * 768]
                        .rearrange("p (s e) -> p s e", s=3),
                        pj["q"][:].rearrange("p (s e) -> p s e", s=3), 1.0,
                        biq_ap.unsqueeze(1).broadcast_to([128, 3, 256]),
                        AL.mult, AL.add)
                    nc.scalar.activation(ktg[:, it * 768:(it + 1) * 768],
                                         pj["k"][:], AF.Copy)
                    nc.scalar.activation(vtg[:, it * 768:(it + 1) * 768],
                                         pj["v"][:], AF.Copy)
                    qv = qtg[:, it * 768:(it + 1) * 768] \
                        .rearrange("p (s e) -> p s e", s=3) \
                        .unsqueeze(2).broadcast_to([128, 3, 3, 256])
                    kv = ktg[:, it * 768:(it + 1) * 768] \
                        .rearrange("p (t e) -> p t e", t=3) \
                        .unsqueeze(1).broadcast_to([128, 3, 3, 256])
                    nc.vector.tensor_tensor(
                        prod[:, it * 2304:(it + 1) * 2304]
                        .rearrange("p (s t e) -> p s t e", s=3, t=3),
                        qv, kv, AL.mult)

            def stage2a(g):
                t0g, gn, ng = GROUPS[g]
                prod = prod_b[g % 2]
                r16, r8 = r16_b[g % 2], r8_b[g % 2]
                halve(r16[:, :ng * 1152], prod[:, :ng * 2304],
                      ng * 72, 32, nc.vector)
                halve(r8[:, :ng * 576], r16[:, :ng * 1152],
                      ng * 72, 16, nc.vector)

            def stage2(g):
                t0g, gn, ng = GROUPS[g]
                r8, r4, r2 = r8_b[g % 2], r4_b[g % 2], r2_b[g % 2]
                sc, exi = sc_b[g % 2], exi_b[g % 2]
                den, dr_t, rr = den_b[g % 2], dr_b[g % 2], rr_b_[g % 2]
                zt, zb = zt_b[g % 2], zb_b_[g % 2]
                halve(r4[:, :ng * 288], r8[:, :ng * 576],
                      ng * 72, 8, nc.gpsimd)
                halve(r2[:, :ng * 144], r4[:, :ng * 288],
                      ng * 72, 4, nc.gpsimd)
                # exp slope A is pre-folded into wi_q/wi_k host-side, so
                # exi = int32(sc + B) straight from the last halve
                halve(sc[:, :ng * 72], r2[:, :ng * 144],
                      ng * 72, 2, nc.gpsimd)
                nc.vector.tensor_scalar(exi[:, :ng * 72], sc[:, :ng * 72],
                                        1.0, EXP_B, AL.mult, AL.add)
                ex = exi[:].bitcast(F32)
                evt = ex[:, :ng * 72].rearrange("p (b e) -> p b e", b=ng * 3)
                dnv = den[:, :ng * 24].rearrange("p (b e) -> p b e", b=ng * 3)
                nc.gpsimd.tensor_tensor(dnv, evt[:, :, 0:8], evt[:, :, 8:16],
                                        AL.add)
                nc.gpsimd.tensor_tensor(dnv, dnv, evt[:, :, 16:24], AL.add)
                nc.vector.reciprocal(dr_t[:, :ng * 24], den[:, :ng * 24])
                wv_b = wimp_ap.unsqueeze(1).unsqueeze(3) \
                    .broadcast_to([128, ng, 3, 8])
                nc.gpsimd.tensor_tensor(
                    rr[:, :ng * 24].rearrange("p (g s h) -> p g s h",
                                              s=3, h=8),
                    dr_t[:, :ng * 24].rearrange("p (g s h) -> p g s h",
                                                s=3, h=8),
                    wv_b, AL.mult)
                rr_b2 = rr[:, :ng * 24] \
                    .rearrange("p (b h) -> p b h", b=ng * 3) \
                    .unsqueeze(2).broadcast_to([128, ng * 3, 3, 8])
                nc.gpsimd.tensor_tensor(
                    zt[:, :ng * 72].rearrange("p (b t h) -> p b t h",
                                              t=3, h=8),
                    ex[:, :ng * 72].rearrange("p (b t h) -> p b t h",
                                              t=3, h=8),
                    rr_b2, AL.mult)
                ztv = zt[:, :ng * 72].rearrange("p (g e) -> p g e", g=ng)
                zbv = zb[:, :ng * 24].rearrange("p (g e) -> p g e", g=ng)
                nc.gpsimd.tensor_tensor(zbv, ztv[:, :, 0:24],
                                        ztv[:, :, 24:48], AL.add)
                nc.gpsimd.tensor_tensor(zbv, zbv, ztv[:, :, 48:72], AL.add)
                vtg = vtg_b[g % 3]
                p3, c01 = p3_b[g % 2], c01_b[g % 2]
                comb = comb_b[g % 2]
                zzb = zb[:, :ng * 24] \
                    .rearrange("p (g t h) -> p g t h", t=3, h=8) \
                    .unsqueeze(3).broadcast_to([128, ng, 3, 32, 8])
                vt_v = vtg[:, :ng * 768] \
                    .rearrange("p (g t d h) -> p g t d h", t=3, d=32, h=8)
                nc.vector.tensor_tensor(
                    p3[:, :ng * 768]
                    .rearrange("p (g t d h) -> p g t d h", t=3, d=32, h=8),
                    zzb, vt_v, AL.mult)
                p3v = p3[:, :ng * 768].rearrange("p (g e) -> p g e", g=ng)
                nc.gpsimd.tensor_tensor(
                    c01[:, :ng * 256].rearrange("p (g e) -> p g e", g=ng),
                    p3v[:, :, 0:256], p3v[:, :, 256:512], AL.add)
                nc.gpsimd.tensor_tensor(
                    comb[:, :ng * 256].rearrange("p (g e) -> p g e", g=ng),
                    c01[:, :ng * 256].rearrange("p (g e) -> p g e", g=ng),
                    p3v[:, :, 512:768], AL.add)

            def stage3(g):
                t0g, gn, ng = GROUPS[g]
                vtg = vtg_b[g % 3]
                comb = comb_b[g % 2]
                ctch = ctch_b[g % 2]
                for it in range(ng):
                    tp = trps.tile([128, 256], BF16, tag="tr", name="tp")
                    for ct in range(2):
                        nc.tensor.transpose(
                            tp[:, ct * 128:(ct + 1) * 128],
                            comb[:, it * 256 + ct * 128:
                                 it * 256 + ct * 128 + 128], id_sb[:])
                    nc.vector.tensor_copy(
                        ctch[:].rearrange("p (c e) -> p c e", c=2)
                        [:, :, it * 128:it * 128 + 128],
                        tp[:].rearrange("p (c e) -> p c e", c=2))
                r0b = 1 + t0g // W
                nrow = gn // W
                for m in range(2):
                    po = ps.tile([128, 512], F32, tag="ps", name="po")
                    lhs = wo_ap.rearrange("p (k m c) -> p k m c",
                                             k=2, m=2)[:, :, m]
                    rhs = ctch[:].rearrange("p (c n) -> p c n", c=2) \
                        [:, :, :gn]
                    nc.tensor.matmul(po[:, :gn], lhs, rhs, start=True,
                                     stop=True, perf_mode=DR)
                    dst = int_buf[:, m * NBP + r0b * PW:
                                  m * NBP + (r0b + nrow) * PW] \
                        .rearrange("p (r w) -> p r w", w=PW)[:, :, 1:1 + W]
                    nc.vector.scalar_tensor_tensor(
                        dst, po[:, :gn].rearrange("p (r w) -> p r w", w=W),
                        1.0 / SW,
                        bo_sb[:, m:m + 1].unsqueeze(1)
                        .broadcast_to([128, nrow, W]),
                        AL.mult, AL.add)
                if g == 0:
                    for m in range(2):
                        nc.vector.tensor_tensor(
                            int_buf[:, m * NBP + PW:m * NBP + 2 * PW],
                            int_buf[:, m * NBP + PW:m * NBP + 2 * PW],
                            mask_sb[:, 0:PW], AL.mult)
                if g == 8:
                    for m in range(2):
                        nc.vector.tensor_tensor(
                            int_buf[:, m * NBP + 34 * PW:
                                    m * NBP + 35 * PW],
                            int_buf[:, m * NBP + 34 * PW:
                                    m * NBP + 35 * PW],
                            mask_sb[:, PW:2 * PW], AL.mult)
                for cci in CONV_AFTER_G.get(g, []):
                    emit_conv_chunk(cci)

            NG = len(GROUPS)
            for sj, pj_ in A_AT[-1]:
                emit_A_unit(sj, pj_)
            for i in range(NG + 2):
                for sj, pj_ in A_AT.get(i, []):
                    emit_A_unit(sj, pj_)
                if 1 <= i <= NG:
                    stage2a(i - 1)
                if i < NG:
                    stage1(i)
                if 1 <= i <= NG:
                    stage2(i - 1)
                if 2 <= i <= NG + 1:
                    stage3(i - 2)

    split_drain_waits(nc)
    return nc


def split_drain_waits(nc):
    """This container's walrus rejects instructions carrying more than one
    (Drain: any) sem wait; hoist excess waits onto preceding single-wait
    no-ops on the same engine (same blocking semantics, in-order queues)."""
    k = 0
    for f in nc.m.functions:
        for b in f.blocks:
            insts = b.instructions
            out, changed = [], False
            for inst in insts:
                si = inst.sync_info
                keep = 0 if inst.opcode == "Drain" else 1
                if si is not None and si.on_wait and len(si.on_wait) > keep:
                    waits = list(si.on_wait)
                    for wchunk in waits[keep:]:
                        nop = mybir.InstNoOp(name=f"waitsplit_{k}", ins=[],
                                             outs=[])
                        k += 1
                        nop.engine = inst.engine
                        nop.sync_info = mybir.SyncInfo(on_wait=[wchunk],
                                                       on_update=[])
                        out.append(nop)
                    inst.sync_info = mybir.SyncInfo(
                        on_wait=list(waits[:keep]),
                        on_update=list(si.on_update))
                    changed = True
                out.append(inst)
            if changed:
                b.instructions = out
    return k


def prep_inputs(x, Wq, bq, bn_q, Wk, bk, bn_k, Wv, bv, bn_v,
                imp_w1, imp_b1, imp_w2, imp_b2,
                attn_in_w, attn_in_b, attn_out_w, attn_out_b,
                op_w1, op_b1, op_bn, op_w2, op_b2):
    """Host-side folding + per-core sharding. Returns list of 8 in_maps."""
    x = np.asarray(x, np.float32)

    # importance gating weights (tiny MLP on pooled means)
    pooled = x.mean(axis=(3, 4)).transpose(1, 0, 2).reshape(B, S * C)
    hgate = _gelu(pooled @ np.asarray(imp_w1, np.float32).T
                  + np.asarray(imp_b1, np.float32))
    logits = hgate @ np.asarray(imp_w2, np.float32).T \
        + np.asarray(imp_b2, np.float32)
    lm = logits.max(axis=1, keepdims=True)
    eg = np.exp(logits - lm)
    wgate = (eg / eg.sum(axis=1, keepdims=True)).astype(np.float32)  # [B,S]

    wq_eff, bq_eff = _fold_bn(np.asarray(Wq, np.float32),
                              np.asarray(bq, np.float32),
                              np.asarray(bn_q, np.float32))
    wk_eff, bk_eff = _fold_bn(np.asarray(Wk, np.float32),
                              np.asarray(bk, np.float32),
                              np.asarray(bn_k, np.float32))
    wv_eff, bv_eff = _fold_bn(np.asarray(Wv, np.float32),
                              np.asarray(bv, np.float32),
                              np.asarray(bn_v, np.float32))

    wq_p = _f8(_lhsT_pack(wq_eff) * SW)
    wk_p = _f8(_lhsT_pack(wk_eff) * SW)
    wv_p = _f8(_lhsT_pack(wv_eff) * SW)

    bq_all = np.zeros((128, 18), np.float32)
    for pi, be in enumerate((bq_eff, bk_eff, bv_eff)):
        for s in range(S):
            for m in range(2):
                bq_all[:, (pi * 3 + s) * 2 + m] = be[s, m * 128:(m + 1) * 128]

    # in-proj: rhs packs [c_in(2m x 128p), cols]; q scaled by 1/sqrt(HD);
    # v output channels permuted to (d, h) order.
    aw = np.asarray(attn_in_w, np.float32)
    ab = np.asarray(attn_in_b, np.float32)
    scale_q = 1.0 / math.sqrt(HD)
    Wiq = aw[:C] * scale_q
    Wik = aw[C:2 * C]
    Wiv = aw[2 * C:]
    biq_s = ab[:C] * scale_q
    biv = ab[2 * C:]
    perm = np.arange(C).reshape(NH, HD).T.reshape(-1)  # c2' = d*8+h
    Wiv_p = Wiv[perm]

    def rhs_pack(wm):  # [cols, c_in] -> [128, 2m x cols]
        ncol = wm.shape[0]
        wt = wm.T.reshape(2, 128, ncol)              # [m, p, cols]
        return np.ascontiguousarray(
            np.transpose(wt, (1, 0, 2)).reshape(128, 2 * ncol))

    # q/k in-proj carry the Schraudolph exp slope: qt*kt sums must equal
    # (2^23/ln2) * score_true, split between the two sides within fp8 range
    mq = max(float(np.abs(Wiq).max()), float(np.abs(biq_s).max()) / 8.0)
    mk = float(np.abs(Wik).max())
    aq = math.sqrt(EXP_A * mk / mq)
    ak = EXP_A / aq
    if aq * mq > 230.0:
        aq = 230.0 / mq
        ak = EXP_A / aq
    if ak * mk > 230.0:
        ak = 230.0 / mk
        aq = EXP_A / ak
    wi_all = _f8(np.concatenate([
        rhs_pack(Wiq) * aq, rhs_pack(Wik) * ak,
        rhs_pack(Wiv_p) * SW], axis=1))

    biq_all = np.broadcast_to(biq_s[None, :] * aq, (128, 256)) \
        .astype(np.float32)

    ow = np.asarray(attn_out_w, np.float32)
    wo_t = ow.T[perm]                                # [c_in', c_out]
    wo_t = wo_t.reshape(2, 128, 2, 128)              # [k, p, m, 128]
    wo_p = _f8(np.ascontiguousarray(
        np.transpose(wo_t, (1, 0, 2, 3)).reshape(128, 512)) * SW)

    cw = np.asarray(op_w1, np.float32)
    g, be_, mu, var = (np.asarray(op_bn, np.float32)[i] for i in range(4))
    sc1 = g / np.sqrt(var + EPS)
    cw_eff = cw * sc1[:, None, None, None]
    cb_eff = (np.asarray(op_b1, np.float32) - mu) * sc1 + be_
    cw_pack = np.zeros((128, 4608), np.float32)
    for tap in range(9):
        dh, dw = tap // 3, tap % 3
        wt = cw_eff[:, :, dh, dw].T.reshape(2, 128, 256)   # [k, p, c_out]
        for k in range(2):
            cw_pack[:, (tap * 2 + k) * 256:(tap * 2 + k + 1) * 256] = wt[k]
    cw_pack = _f8(cw_pack * SW)
    cb_all = np.stack([cb_eff[:128], cb_eff[128:]], axis=1).astype(np.float32)

    w2m = np.asarray(op_w2, np.float32)
    w2_t = w2m.T.reshape(2, 128, 2, 128)
    w2_p = _f8(np.ascontiguousarray(
        np.transpose(w2_t, (1, 0, 2, 3)).reshape(128, 512)) * SW)
    b2v = np.asarray(op_b2, np.float32)
    b2_all = np.stack([b2v[:128], b2v[128:]], axis=1).astype(np.float32)

    ident = np.eye(128, dtype=BF)

    # out-proj bias including folded v bias: (bout + Wout @ biv) * wsum
    bo_base = np.asarray(attn_out_b, np.float32) + ow @ biv

    in_maps = []
    for core in range(8):
        b, half = core // 2, core % 2
        h0 = 32 * half
        xs = np.zeros((S, C, RH, W), np.float32)
        if half == 0:
            xs[:, :, 1:34] = x[:, b, :, 0:33]
        else:
            xs[:, :, 0:33] = x[:, b, :, 31:64]
        # device layout [128p, (s, k, tok)]
        xs_r = np.ascontiguousarray(
            xs.reshape(S, 2, 128, RH * W).transpose(2, 0, 1, 3)
            .reshape(128, S * 2 * RH * W))
        xs_r = _f8(xs_r * SX)

        # residual (+ b2) on the 66-wide padded grid, packed [128, 2*2112]
        xrow = x[1, b].reshape(2, 128, 64, 64)[:, :, h0:h0 + 32, :]
        xres = np.zeros((2, 128, 32, PW), np.float32)
        xres[:, :, :, 1:65] = xrow
        xres += b2_all.T.reshape(2, 128, 1, 1)
        xres = np.ascontiguousarray(
            xres.transpose(1, 0, 2, 3).reshape(128, 2 * 32 * PW))

        mask = np.ones((128, 132), np.float32)
        if half == 0:
            mask[:, 0:PW] = 0.0       # buf row 1 (token row 0) is zero-pad
        else:
            mask[:, PW:2 * PW] = 0.0  # buf row 34 (token row 33) is zero-pad
        mask = mask.astype(F8)

        wimp = np.broadcast_to(wgate[b][None, :], (128, 3)).astype(np.float32)
        wsum = float(wgate[b].sum())
        bo_eff = bo_base * wsum * SW
        bo_all = np.stack([bo_eff[:128], bo_eff[128:]], axis=1) \
            .astype(np.float32)

        wallA = np.concatenate(
            [wq_p.view(np.uint8), wk_p.view(np.uint8), wv_p.view(np.uint8)],
            axis=1).view(F8)
        wallB = np.concatenate(
            [wi_all.view(np.uint8), wo_p.view(np.uint8),
             cw_pack.view(np.uint8), w2_p.view(np.uint8),
             np.ascontiguousarray(mask).view(np.uint8)],
            axis=1).view(F8)
        fconsts = np.ascontiguousarray(np.concatenate(
            [bq_all, biq_all, bo_all, cb_all, wimp], axis=1)
            .astype(np.float32))

        in_maps.append({
            "xs": xs_r, "wallA": np.ascontiguousarray(wallA),
            "wallB": np.ascontiguousarray(wallB),
            "fconsts": fconsts, "xres": xres, "ident": ident,
        })
    return in_maps


_NC_CACHE = None


def kernel(**inputs):
    global _NC_CACHE, LAST_RESULT
    params = {k: v for k, v in inputs.items() if k not in ("target_h",
                                                           "target_w")}
    in_maps = prep_inputs(**params)
    if _NC_CACHE is None:
        _NC_CACHE = build_program()
    nc = _NC_CACHE
    res = bass_utils.run_bass_kernel_spmd(nc, in_maps, core_ids=list(range(8)))
    LAST_RESULT = res
    out = np.zeros((B, C, H, W), np.float32)
    for core in range(8):
        b, half = core // 2, core % 2
        y = res.results[core]["yout"].reshape(2, 128, 32, W)
        out[b, :128, 32 * half:32 * half + 32] = y[0]
        out[b, 128:, 32 * half:32 * half + 32] = y[1]
    return out
